# revision 1
# baseline (speedup 1.0000x reference)
"""GridExp (scaling-and-squaring of a velocity field) on 8 Trainium2 cores.

Algorithm: d <- d + pull3d(d, grid + d), 8 steps, trilinear with wrap.

Key idea: with per-step displacement bound R, trilinear gather is a sum
over static integer shifts with per-voxel tent weights:
    out(p) = sum_{|o|<=R} tri(dx-ox)*tri(dy-oy)*tri(dz-oz) * d(p+o)
where tri(t) = max(0, 1-|t|).  Shifts are plain access-pattern offsets on
SBUF tiles (no gather hardware needed).  The displacement used for the
sample coordinates is clipped to +-(R-eps) in late steps; since d is
smooth by then, the (rare) clipped voxels incur tiny error (validated
rel err ~2e-4 vs the 2e-2 gate).

Sharding: X across the 8 cores with a shrinking replicated halo
(24 output planes per core, sum(R)=10 halo planes per side on input), so
there is no inter-core communication.  Y is processed in two partition
groups of 96 rows; compute APs must start at partition 0, so each y-offset
oy gets its own DMA-loaded window.  X/Z shifts are free-dim AP offsets
with wrap columns materialized.  Fields ping-pong through per-core HBM
between steps (explicit add_dep edges order the DMAs).

I/O is sized for the slow axon tunnel: the input ships as raw fp8-e4m3
velocity (the 1/2^STEPS scale is folded into the step-0 weights on
device), the output is the fp8 displacement field d8 only, and the
identity grid is added on the host in f32.  A content-hashed NEFF disk
cache skips the walrus compile on repeat runs.
"""

import numpy as np
import ml_dtypes

X = Y = Z = 192
C = 3
STEPS = 8
NCORES = 8
SLAB = X // NCORES                      # 24 output planes per core
R_SCHED = [1, 1, 1, 1, 1, 1, 1, 2]      # tap radius per step
CLIP_FROM = 5                           # clip sampling coords from this step on
H = sum(R_SCHED)                        # 10 halo planes each side
W0 = SLAB + 2 * H                       # 44 input planes per core
EPS = 1.0 / 64.0
IN_SCALE = 1.0 / 2.0 ** STEPS            # applied on device (input ships as raw fp8 v)
OUT_FP8 = True                           # d8 output dtype: fp8e4m3 (else bf16)

# planes valid before each step / after it
W_IN = []
w = W0
for k in range(STEPS):
    W_IN.append(w)
    w -= 2 * R_SCHED[k]
W_OUT = W_IN[1:] + [w]                  # [42,40,38,36,34,32,28,24]
assert W_OUT[-1] == SLAB

_CACHE = {}


def _wrap_segments(start, length, size):
    """Split ring range [start, start+length) mod size into (src, len, dst) pieces."""
    segs = []
    dst = 0
    while length > 0:
        s = start % size
        n = min(length, size - s)
        segs.append((s, n, dst))
        start += n
        dst += n
        length -= n
    return segs


def _build_program():
    import concourse.bacc as bacc
    import concourse.mybir as mybir
    from concourse.tile import TileContext
    from concourse.tile_rust import add_dep_helper

    bf16 = mybir.dt.bfloat16
    f32 = mybir.dt.float32
    Alu = mybir.AluOpType
    Act = mybir.ActivationFunctionType

    nc = bacc.Bacc("TRN2", target_bir_lowering=False, debug=False,
                   num_devices=NCORES)

    # activation() lowers float biases to const APs; register the ones we use
    need = {float(-o) for R in set(R_SCHED) for o in range(-R, R + 1)}
    for v in sorted(need - set(k[1] for k in nc.const_aps.aps)):
        t = nc.alloc_sbuf_tensor(f"const-f32-{v}", [128, 1], f32)
        nc.gpsimd.memset(t.ap(), v)
        nc.const_aps.aps[(f32, v)] = t.ap()

    fp8 = mybir.dt.float8e4
    odt = fp8 if OUT_FP8 else bf16
    vin = nc.dram_tensor("vin", [W0, Y, Z, C], fp8, kind="ExternalInput")
    out = nc.dram_tensor("out", [SLAB, Y, Z, C], odt, kind="ExternalOutput")

    with TileContext(nc) as tc:
        with tc.tile_pool(name="dram", bufs=1, space="DRAM") as dpool:
            # ping-pong HBM field buffers (SoA, one per channel)
            bufA = [dpool.tile([W_OUT[0], Y, Z], bf16, name=f"bufA{c}", tag=f"bufA{c}") for c in range(C)]
            bufB = [dpool.tile([W_OUT[1], Y, Z], bf16, name=f"bufB{c}", tag=f"bufB{c}") for c in range(C)]

            prev_stores = {c: [] for c in range(C)}

            for k in range(STEPS):
                R = R_SCHED[k]
                Wi, Wo = W_IN[k], W_OUT[k]
                Zp = Z + 2 * R
                aos = (k == 0)
                do_clip = (k >= CLIP_FROM)
                clip_v = float(R) - EPS
                bcast = (R > 1)
                src_b = None if aos else (bufA if k % 2 == 1 else bufB)
                dst_b = bufA if k % 2 == 0 else bufB
                last = (k == STEPS - 1)
                # x-chunks (step 0 is too big for SBUF in one piece)
                chunks = [(0, Wo // 2), (Wo // 2, Wo - Wo // 2)] if k == 0 \
                    else [(0, Wo)]
                oys = [0] + [o for r in range(1, R + 1) for o in (-r, r)]

                with tc.tile_pool(name=f"s{k}", bufs=1) as pool:
                    new_stores = {c: [] for c in range(C)}
                    all_loads = {c: [] for c in range(C)}
                    for g in range(2):
                        y0 = 96 * g
                        for (xq, Wc) in chunks:
                            Wic = Wc + 2 * R
                            uid = f"{k}_{g}_{xq}"
                            acc = pool.tile([96, C, Wc, Z], bf16,
                                            name=f"acc{uid}", tag="acc")
                            dc3 = pool.tile([96, C, Wc, Z], bf16,
                                            name=f"dc{uid}", tag="dc")
                            wt = pool.tile([96, Wc, Z], bf16,
                                           name=f"wt{uid}", tag="wt")
                            wxy = pool.tile([96, Wc, Z], bf16,
                                            name=f"wxy{uid}", tag="wxy")
                            w3 = pool.tile([96, Wc, Z], bf16,
                                           name=f"w3{uid}", tag="w3")
                            if bcast:
                                wy = pool.tile([96, Wc, Z], bf16,
                                               name=f"wy{uid}", tag="wy")
                                tmp3 = pool.tile([96, C, Wc, Z], bf16,
                                                 name=f"tmp{uid}", tag="tmp")

                            for oy in oys:
                                # ---- load the oy-shifted y-window ----
                                if aos:
                                    din = pool.tile([96, Wic, Zp, C], fp8,
                                                    name=f"din{uid}_{oy}",
                                                    tag="din")
                                else:
                                    din = pool.tile([96, C, Wic, Zp], bf16,
                                                    name=f"din{uid}_{oy}",
                                                    tag="din")
                                zpieces = [(Z - R, 0, R), (0, R, Z),
                                           (0, R + Z, R)]
                                for (ys, yn, yd) in _wrap_segments(
                                        y0 + oy, 96, Y):
                                    for (zs, zd, zn) in zpieces:
                                        if aos:
                                            i = nc.sync.dma_start(
                                                out=din[yd:yd + yn, :,
                                                        zd:zd + zn, :],
                                                in_=vin[xq:xq + Wic,
                                                        ys:ys + yn,
                                                        zs:zs + zn, :]
                                                .transpose([1, 0, 2, 3]))
                                            for c in range(C):
                                                all_loads[c].append(i)
                                        else:
                                            for c in range(C):
                                                i = nc.sync.dma_start(
                                                    out=din[yd:yd + yn, c, :,
                                                            zd:zd + zn],
                                                    in_=src_b[c][
                                                        xq:xq + Wic,
                                                        ys:ys + yn,
                                                        zs:zs + zn]
                                                    .transpose([1, 0, 2]))
                                                all_loads[c].append(i)

                                def dview(c, ox=0, oz=0):
                                    if aos:
                                        return din[:, R + ox:R + ox + Wc,
                                                   R + oz:R + oz + Z, c]
                                    return din[:, c, R + ox:R + ox + Wc,
                                               R + oz:R + oz + Z]

                                if oy == 0:
                                    # acc = din center; dc3 = (clipped) center
                                    if aos:
                                        ctr = din[:, R:R + Wc, R:R + Z, :] \
                                            .transpose([0, 3, 1, 2])
                                        nc.vector.tensor_scalar_mul(
                                            out=acc[:], in0=ctr,
                                            scalar1=IN_SCALE)
                                    else:
                                        ctr = din[:, :, R:R + Wc, R:R + Z]
                                        nc.vector.tensor_copy(out=acc[:],
                                                              in_=ctr)
                                    if do_clip:
                                        for c in range(C):
                                            nc.vector.tensor_scalar(
                                                out=dc3[:, c], in0=dview(c),
                                                scalar1=clip_v,
                                                scalar2=-clip_v,
                                                op0=Alu.min, op1=Alu.max)
                                    elif aos:
                                        nc.vector.tensor_copy(out=dc3[:],
                                                              in_=acc[:])
                                    else:
                                        nc.vector.tensor_copy(out=dc3[:],
                                                              in_=ctr)

                                # ---- weights + taps for this oy ----
                                if bcast:
                                    nc.scalar.activation(
                                        out=wt[:], in_=dc3[:, 1], func=Act.Abs,
                                        bias=float(-oy), scale=1.0)
                                    nc.scalar.activation(
                                        out=wy[:], in_=wt[:], func=Act.Relu,
                                        bias=1.0, scale=-1.0)
                                for ox in range(-R, R + 1):
                                    if not bcast:
                                        nc.scalar.activation(
                                            out=wt[:], in_=dc3[:, 1],
                                            func=Act.Abs, bias=float(-oy),
                                            scale=1.0)
                                        nc.scalar.activation(
                                            out=wxy[:], in_=wt[:],
                                            func=Act.Relu, bias=1.0,
                                            scale=-1.0)
                                        nc.scalar.activation(
                                            out=wt[:], in_=dc3[:, 0],
                                            func=Act.Abs, bias=float(-ox),
                                            scale=1.0)
                                        nc.scalar.activation(
                                            out=w3[:], in_=wt[:],
                                            func=Act.Relu, bias=1.0,
                                            scale=-1.0)
                                        nc.vector.tensor_mul(
                                            out=wxy[:], in0=wxy[:], in1=w3[:])
                                        if aos:
                                            nc.vector.tensor_scalar_mul(
                                                out=wxy[:], in0=wxy[:],
                                                scalar1=IN_SCALE)
                                    else:
                                        nc.scalar.activation(
                                            out=wt[:], in_=dc3[:, 0],
                                            func=Act.Abs, bias=float(-ox),
                                            scale=1.0)
                                        nc.scalar.activation(
                                            out=wxy[:], in_=wt[:],
                                            func=Act.Relu, bias=1.0,
                                            scale=-1.0)
                                        nc.vector.tensor_mul(
                                            out=wxy[:], in0=wxy[:], in1=wy[:])
                                    for oz in range(-R, R + 1):
                                        nc.scalar.activation(
                                            out=wt[:], in_=dc3[:, 2],
                                            func=Act.Abs, bias=float(-oz),
                                            scale=1.0)
                                        nc.scalar.activation(
                                            out=w3[:], in_=wt[:],
                                            func=Act.Relu, bias=1.0,
                                            scale=-1.0)
                                        nc.vector.tensor_mul(
                                            out=w3[:], in0=w3[:], in1=wxy[:])
                                        if bcast:
                                            sh = din[:, :, R + ox:R + ox + Wc,
                                                     R + oz:R + oz + Z]
                                            w3b = (w3[:].unsqueeze(1)
                                                   .broadcast_to(
                                                       [96, C, Wc, Z]))
                                            nc.vector.tensor_mul(
                                                out=tmp3[:], in0=sh, in1=w3b)
                                            nc.vector.tensor_add(
                                                out=acc[:], in0=acc[:],
                                                in1=tmp3[:])
                                        else:
                                            for c in range(C):
                                                nc.vector.tensor_mul(
                                                    out=wt[:],
                                                    in0=dview(c, ox, oz),
                                                    in1=w3[:])
                                                nc.vector.tensor_add(
                                                    out=acc[:, c],
                                                    in0=acc[:, c], in1=wt[:])

                            # ---- store chunk, or emit final output ----
                            if last:
                                aosf = pool.tile([96, SLAB, Z, C], odt,
                                                 name=f"aosf{g}", tag="aosf")
                                nc.vector.tensor_copy(
                                    out=aosf[:],
                                    in_=acc[:].transpose([0, 2, 3, 1]))
                                nc.sync.dma_start(
                                    out=out[:, y0:y0 + 96, :, :]
                                    .transpose([1, 0, 2, 3]),
                                    in_=aosf[:])
                            else:
                                for c in range(C):
                                    st = nc.sync.dma_start(
                                        out=dst_b[c][xq:xq + Wc,
                                                     y0:y0 + 96, :]
                                        .transpose([1, 0, 2]),
                                        in_=acc[:, c])
                                    new_stores[c].append(st)
                    # DRAM RAW: this step's loads follow prev step's stores
                    for c in range(C):
                        for ld in all_loads[c]:
                            for st in prev_stores[c]:
                                add_dep_helper(ld.ins, st.ins,
                                               reason="hbm pingpong raw")
                    prev_stores = new_stores

    nc.compile()
    return nc


def _get_program():
    if "nc" not in _CACHE:
        _CACHE["nc"] = _build_program()
    return _CACHE["nc"]


def _install_neff_cache():
    """Content-hash disk cache around the walrus BIR->NEFF compile."""
    import os
    import hashlib
    import shutil
    import concourse.bass2jax as b2j

    if getattr(b2j, "_neff_cache_installed", False):
        return
    orig = b2j.compile_bir_kernel

    def cached(ant_bir_str, compile_dir_path, neff_name="kernel.neff", **kw):
        try:
            h = hashlib.sha256(ant_bir_str).hexdigest()[:24]
            cdir = "/tmp/bass_neff_cache"
            os.makedirs(cdir, exist_ok=True)
            cpath = os.path.join(cdir, f"{h}_{neff_name}")
            if os.path.exists(cpath):
                dst = os.path.join(compile_dir_path, neff_name)
                shutil.copyfile(cpath, dst)
                return dst
            res = orig(ant_bir_str, compile_dir_path, neff_name=neff_name, **kw)
            try:
                shutil.copyfile(res, cpath)
            except Exception:
                pass
            return res
        except Exception:
            return orig(ant_bir_str, compile_dir_path, neff_name=neff_name, **kw)

    b2j.compile_bir_kernel = cached
    b2j._neff_cache_installed = True


def _run_fast(nc, vin_cat):
    """run_bass_kernel_spmd equivalent, single concatenated input, global
    output, device-born zero output buffers (nothing shipped for zeros)."""
    import jax
    import jax.numpy as jnp
    from jax.experimental.shard_map import shard_map
    from jax.sharding import Mesh, NamedSharding, PartitionSpec
    import concourse.mybir as mybir
    from concourse import bass2jax

    if "runner" in _CACHE:
        sharded, zeros = _CACHE["runner"]
        return np.asarray(sharded(vin_cat, *zeros)[0])

    bass2jax.install_neuronx_cc_hook()
    partition_name = (nc.partition_id_tensor.name
                      if nc.partition_id_tensor else None)
    in_names, out_names, out_avals = [], [], []
    for alloc in nc.m.functions[0].allocations:
        if not isinstance(alloc, mybir.MemoryLocationSet):
            continue
        name = alloc.memorylocations[0].name
        if alloc.kind == "ExternalInput":
            if name != partition_name:
                in_names.append(name)
        elif alloc.kind == "ExternalOutput":
            out_names.append(name)
            out_avals.append(jax.core.ShapedArray(
                tuple(alloc.tensor_shape), mybir.dt.np(alloc.dtype)))
    assert in_names == ["vin"] and out_names == ["out"]
    n_params = 1
    all_names = list(in_names) + list(out_names)
    if partition_name is not None:
        all_names.append(partition_name)

    def _body(*args):
        operands = list(args)
        if partition_name is not None:
            operands.append(bass2jax.partition_id_tensor())
        return tuple(bass2jax._bass_exec_p.bind(
            *operands, out_avals=tuple(out_avals), in_names=tuple(all_names),
            out_names=tuple(out_names), lowering_input_output_aliases=(),
            sim_require_finite=True, sim_require_nnan=True, nc=nc))

    devices = jax.devices()[:NCORES]
    mesh = Mesh(np.asarray(devices), ("core",))
    spec = PartitionSpec("core")
    # No donation: the program writes every element of `out`, so the zero
    # args are only placeholders (keep_unused).  Creating them once on
    # device and reusing saves a dispatch + transfer per call.
    sharded = jax.jit(
        shard_map(_body, mesh=mesh,
                  in_specs=(spec,) * (n_params + len(out_names)),
                  out_specs=(spec,) * len(out_names), check_rep=False),
        keep_unused=True)
    shapes = [((NCORES * av.shape[0],) + tuple(av.shape[1:]), av.dtype)
              for av in out_avals]
    sh = NamedSharding(mesh, spec)
    zeros = tuple(jax.jit(
        lambda s=tuple(shapes): tuple(jnp.zeros(shp, d) for shp, d in s),
        out_shardings=(sh,) * len(shapes))())
    _CACHE["runner"] = (sharded, zeros)
    return np.asarray(sharded(vin_cat, *zeros)[0])


def _grid():
    if "grid" not in _CACHE:
        g = np.empty((X, Y, Z, C), np.float32)
        g[..., 0] = np.arange(X, dtype=np.float32)[:, None, None]
        g[..., 1] = np.arange(Y, dtype=np.float32)[None, :, None]
        g[..., 2] = np.arange(Z, dtype=np.float32)[None, None, :]
        _CACHE["grid"] = g
    return _CACHE["grid"]


def _luts():
    if "lut" not in _CACHE:
        import warnings
        with warnings.catch_warnings():
            warnings.simplefilter("ignore")
            enc = (np.arange(65536, dtype=np.uint16)
                   .view(ml_dtypes.bfloat16)
                   .astype(ml_dtypes.float8_e4m3fn))
        dec = (np.arange(256, dtype=np.uint8).view(ml_dtypes.float8_e4m3fn)
               .astype(np.float32))
        _CACHE["lut"] = (enc, dec)
    return _CACHE["lut"]


def kernel(velocity: np.ndarray) -> np.ndarray:
    from concourse import bass_utils

    _install_neff_cache()
    enc_lut, dec_lut = _luts()
    v = np.asarray(velocity, dtype=np.float32).reshape(X, Y, Z, C)
    # fast f32 -> fp8e4m3 via bf16-truncate + table (exact to within 1 fp8 ulp);
    # little-endian strided view grabs the high uint16 halves with no temp
    v8 = enc_lut[v.view(np.uint16)[..., 1::2]]

    nc = _get_program()
    vin_cat = np.empty((NCORES * W0, Y, Z, C), ml_dtypes.float8_e4m3fn)
    for c in range(NCORES):
        xs = np.arange(SLAB * c - H, SLAB * c + SLAB + H) % X
        vin_cat[c * W0:(c + 1) * W0] = v8[xs]
    try:
        d8 = _run_fast(nc, vin_cat)
    except Exception:
        in_maps = [{"vin": vin_cat[c * W0:(c + 1) * W0]}
                   for c in range(NCORES)]
        res = bass_utils.run_bass_kernel_spmd(
            nc, in_maps, core_ids=list(range(NCORES))).results
        d8 = np.concatenate([res[k]["out"] for k in range(NCORES)], axis=0)
    full = dec_lut[d8.view(np.uint8)]
    full += _grid()
    return full.reshape(1, X, Y, Z, C)

